# revision 18
# baseline (speedup 1.0000x reference)
"""MoE layer (top-2 of 8 experts) on 8 Trainium2 NeuronCores, expert-parallel.

v4: restructured for real-HW matmul cost (stationary weight loads are NOT
hidden: ~125ns per reload, so each stationary must serve >=1024 moving
columns):
 - routing uses host-pretransposed xT; gate logits computed as
   [8 experts x tokens] with gate weights stationary (no PE transposes of x),
   then tiny [8->128] transposes recover per-token layout.
 - FFN w1/w3: hc-outer matmul pairs, each stationary serves 1024 cols.
 - FFN w2: per (hT-block, token-tile) stationary serves both 512-col H
   halves; w2 fully resident in SBUF (loaded once).
 - compaction: combine weights scattered as single bf16 local_scatter, then
   a second round-robin re-deal + re-scatter rebalances partitions
   (peak per-partition count 157 -> 142), shrinking the padded compact
   token count C from 2560 to 2304 (-10% FFN matmul work).
Routing math stays fp32 (exact top-2 selection); FFN is bf16 with fp32
PSUM accumulation.
"""

import os

os.environ.setdefault("JAX_PLATFORMS", "")

import numpy as np

T, H, F, E = 8192, 1024, 4096, 8
P = 128
NCORES = 8
K = 160                      # pass-1 per-partition slot capacity ([16, 512] layout)
K2 = 144                     # pass-2 capacity after round-robin rebalance
C = 16 * K2                  # 2304 compact slots per expert
CT = C // P                  # 18 slot tiles
CHUNKS = [1024, 1024, 256]   # FFN token chunks
assert sum(CHUNKS) == C
NTILE = T // P               # 64 routing tiles
NBLK = 16                    # routing blocks of 512 tokens
HC = H // P                  # 8 h-blocks
FT = F // P                  # 32 f-blocks

_cache: dict = {}


def _build_nc(reps=1):
    import concourse.mybir as mybir
    import concourse.tile as tile
    from concourse import bacc
    from concourse.bass import IndirectOffsetOnAxis
    from concourse.masks import make_identity

    dt = mybir.dt
    Alu = mybir.AluOpType
    Act = mybir.ActivationFunctionType

    nc = bacc.Bacc("TRN2", target_bir_lowering=False)

    xt_in = nc.dram_tensor("xt", [NBLK, P, HC, 512], dt.float32, kind="ExternalInput")
    xb_in = nc.dram_tensor("xb", [T, H], dt.bfloat16, kind="ExternalInput")
    gwt_in = nc.dram_tensor("gwt", [P, HC, E], dt.float32, kind="ExternalInput")
    esel_in = nc.dram_tensor("esel", [P, E], dt.float32, kind="ExternalInput")
    w1_in = nc.dram_tensor("w1t", [FT, P, HC, P], dt.bfloat16, kind="ExternalInput")
    w3_in = nc.dram_tensor("w3t", [FT, P, HC, P], dt.bfloat16, kind="ExternalInput")
    w2_in = nc.dram_tensor("w2n", [FT, P, H], dt.bfloat16, kind="ExternalInput")

    y_out = nc.dram_tensor("y", [C, H], dt.float32, kind="ExternalOutput")
    idx_out = nc.dram_tensor("idx", [C], dt.int32, kind="ExternalOutput")

    with tile.TileContext(nc) as tc:
      for _rep in range(reps):
        with (
            tc.tile_pool(name="const", bufs=1) as cp,
            tc.tile_pool(name="dram", bufs=1, space="DRAM") as dp,
        ):
            identb = cp.tile([P, P], dt.bfloat16)
            make_identity(nc, identb)
            ident8 = cp.tile([8, 8], dt.float32)
            make_identity(nc, ident8)
            gwt = cp.tile([P, HC, E], dt.float32)
            nc.sync.dma_start(gwt[:], gwt_in[:])
            esel = cp.tile([P, E], dt.float32)
            nc.sync.dma_start(esel[:], esel_in[:])

            # w2 resident for the whole FFN: loaded once, early (overlaps
            # routing). [p, fb, h]
            w2r = cp.tile([P, FT, H], dt.bfloat16)
            for fb in range(FT):
                nc.scalar.dma_start(w2r[:, fb, :], w2_in[fb])

            # routing result: cw per token, layout [p, i] -> t = i*128+p
            cw_all = cp.tile([P, NTILE], dt.float32)

            # ---------------- routing (all 8192 tokens) ----------------
            with (
                tc.tile_pool(name="rt_x", bufs=3) as rx,
                tc.tile_pool(name="rt_misc", bufs=3) as rm,
                tc.tile_pool(name="ps_rt", bufs=1, space="PSUM") as pr,
            ):
                for b in range(NBLK):
                    xtb = rx.tile([P, HC, 512], dt.float32, tag="xtb")
                    nc.sync.dma_start(xtb[:], xt_in[b])
                    pg = pr.tile([8, 512], dt.float32, tag="pg", bufs=2)
                    for hc in range(HC):
                        nc.tensor.matmul(
                            pg[:], gwt[:, hc, :], xtb[:, hc, :],
                            start=(hc == 0), stop=(hc == HC - 1),
                        )
                    lgE = rm.tile([8, 512], dt.float32, tag="lgE")
                    nc.vector.tensor_copy(lgE[:], pg[:])
                    for s in range(4):
                        i = b * 4 + s  # 128-token tile index
                        tr = pr.tile([P, 8], dt.float32, tag="tr", bufs=2)
                        nc.tensor.transpose(
                            tr[:], lgE[:, s * P : (s + 1) * P], ident8[:]
                        )
                        mx = rm.tile([P, 8], dt.float32, tag="mx")
                        nc.vector.max(mx[:], tr[:])
                        negs = rm.tile([P, 1], dt.float32, tag="negs")
                        nc.vector.tensor_tensor(
                            negs[:], mx[:, 0:1], mx[:, 1:2], op=Alu.add
                        )
                        nc.vector.tensor_scalar_mul(negs[:], negs[:], -1.0)
                        sig = rm.tile([P, E], dt.float32, tag="sig")
                        nc.scalar.activation(
                            sig[:], tr[:], Act.Sigmoid, bias=negs[:], scale=2.0
                        )
                        msk = rm.tile([P, E], dt.float32, tag="msk")
                        nc.vector.tensor_scalar(
                            msk[:], tr[:], mx[:, 1:2], None, op0=Alu.is_ge
                        )
                        cw8 = rm.tile([P, E], dt.float32, tag="cw8")
                        nc.vector.tensor_tensor(cw8[:], sig[:], msk[:], op=Alu.mult)
                        nc.vector.tensor_tensor(cw8[:], cw8[:], esel[:], op=Alu.mult)
                        nc.vector.tensor_reduce(
                            cw_all[:, i : i + 1], cw8[:],
                            axis=mybir.AxisListType.X, op=Alu.add,
                        )

            # -------- compaction: [16,512] layout, scan + local_scatter --------
            ids128 = cp.tile([P, CT], dt.float32)
            cw128 = cp.tile([P, CT], dt.float32)
            idx_i = cp.tile([P, CT], dt.int32)
            idg_i = cp.tile([P, CT], dt.int32)
            with tc.tile_pool(name="cmp", bufs=1) as sm:
                cwflat = dp.tile([T], dt.float32)
                nc.sync.dma_start(cwflat[:].rearrange("(i p) -> p i", p=P), cw_all[:])
                cw16 = sm.tile([16, 512], dt.float32)
                nc.sync.dma_start(cw16[:], cwflat[:].rearrange("(p f) -> p f", p=16))

                mask16 = sm.tile([16, 512], dt.float32)
                nc.vector.tensor_scalar(mask16[:], cw16[:], 0.0, None, op0=Alu.is_gt)
                zeros16 = sm.tile([16, 512], dt.float32)
                nc.vector.memset(zeros16[:], 0.0)
                scn = sm.tile([16, 512], dt.float32)
                nc.vector.tensor_tensor_scan(
                    scn[:], mask16[:], zeros16[:], 0.0, Alu.add, Alu.add
                )
                pos = sm.tile([16, 512], dt.float32)
                nc.vector.tensor_tensor(pos[:], scn[:], mask16[:], op=Alu.subtract)
                inb = sm.tile([16, 512], dt.float32)
                nc.vector.tensor_scalar(inb[:], pos[:], float(K - 1), None, op0=Alu.is_le)
                sel = sm.tile([16, 512], dt.float32)
                nc.vector.tensor_tensor(sel[:], mask16[:], inb[:], op=Alu.mult)
                posf = sm.tile([16, 512], dt.float32)
                nc.vector.tensor_tensor(posf[:], pos[:], sel[:], op=Alu.mult)
                selm1 = sm.tile([16, 512], dt.float32)
                nc.vector.tensor_scalar(selm1[:], sel[:], 1.0, None, op0=Alu.subtract)
                nc.vector.tensor_tensor(posf[:], posf[:], selm1[:], op=Alu.add)
                posi = sm.tile([16, 512], dt.int16)
                nc.vector.tensor_copy(posi[:], posf[:])

                iop1 = sm.tile([16, 512], dt.int32)
                nc.gpsimd.iota(iop1[:], pattern=[[1, 512]], base=1, channel_multiplier=512)
                idsp1 = sm.tile([16, 512], dt.uint16)
                nc.vector.tensor_copy(idsp1[:], iop1[:])

                # cw as bf16 (single 16-bit scatter; 0.4% rel quantization)
                cwb = sm.tile([16, 512], dt.bfloat16)
                nc.vector.tensor_copy(cwb[:], cw16[:])

                pc_id = sm.tile([16, K], dt.uint16)
                nc.gpsimd.local_scatter(pc_id[:], idsp1[:], posi[:], 16, K, 512)
                pc_cw = sm.tile([16, K], dt.uint16)
                nc.gpsimd.local_scatter(
                    pc_cw[:], cwb[:].bitcast(dt.uint16), posi[:], 16, K, 512
                )

                # ---- pass 2: round-robin re-deal across partitions, then
                # re-compact to K2 (peak per-partition count drops 157->~142
                # because each partition now samples every 16th slot).
                idflat1 = dp.tile([16 * K], dt.uint16)
                nc.sync.dma_start(
                    idflat1[:].rearrange("(p f) -> p f", p=16), pc_id[:]
                )
                cwflat1 = dp.tile([16 * K], dt.uint16)
                nc.sync.dma_start(
                    cwflat1[:].rearrange("(p f) -> p f", p=16), pc_cw[:]
                )
                id_B = sm.tile([16, K], dt.uint16)
                nc.sync.dma_start(id_B[:], idflat1[:].rearrange("(f p) -> p f", p=16))
                cw_B = sm.tile([16, K], dt.uint16)
                nc.sync.dma_start(cw_B[:], cwflat1[:].rearrange("(f p) -> p f", p=16))

                idf2 = sm.tile([16, K], dt.float32)
                nc.vector.tensor_copy(idf2[:], id_B[:])
                vm2 = sm.tile([16, K], dt.float32)
                nc.vector.tensor_scalar(vm2[:], idf2[:], 0.0, None, op0=Alu.is_gt)
                z2 = sm.tile([16, K], dt.float32)
                nc.vector.memset(z2[:], 0.0)
                scn2 = sm.tile([16, K], dt.float32)
                nc.vector.tensor_tensor_scan(
                    scn2[:], vm2[:], z2[:], 0.0, Alu.add, Alu.add
                )
                pos2 = sm.tile([16, K], dt.float32)
                nc.vector.tensor_tensor(pos2[:], scn2[:], vm2[:], op=Alu.subtract)
                inb2 = sm.tile([16, K], dt.float32)
                nc.vector.tensor_scalar(
                    inb2[:], pos2[:], float(K2 - 1), None, op0=Alu.is_le
                )
                sel2 = sm.tile([16, K], dt.float32)
                nc.vector.tensor_tensor(sel2[:], vm2[:], inb2[:], op=Alu.mult)
                posf2 = sm.tile([16, K], dt.float32)
                nc.vector.tensor_tensor(posf2[:], pos2[:], sel2[:], op=Alu.mult)
                sm2 = sm.tile([16, K], dt.float32)
                nc.vector.tensor_scalar(sm2[:], sel2[:], 1.0, None, op0=Alu.subtract)
                nc.vector.tensor_tensor(posf2[:], posf2[:], sm2[:], op=Alu.add)
                posi2 = sm.tile([16, K], dt.int16)
                nc.vector.tensor_copy(posi2[:], posf2[:])

                pc_id2 = sm.tile([16, K2], dt.uint16)
                nc.gpsimd.local_scatter(pc_id2[:], id_B[:], posi2[:], 16, K2, K)
                pc_cw2 = sm.tile([16, K2], dt.uint16)
                nc.gpsimd.local_scatter(pc_cw2[:], cw_B[:], posi2[:], 16, K2, K)

                idfK = sm.tile([16, K2], dt.float32)
                nc.vector.tensor_copy(idfK[:], pc_id2[:])
                zt = sm.tile([16, K2], dt.float32)
                nc.vector.tensor_scalar(
                    zt[:], idfK[:], 0.0, 8193.0, op0=Alu.is_equal, op1=Alu.mult
                )
                nc.vector.tensor_tensor(idfK[:], idfK[:], zt[:], op=Alu.add)
                nc.vector.tensor_scalar(idfK[:], idfK[:], 1.0, None, op0=Alu.subtract)

                idxflat = dp.tile([C], dt.float32)
                nc.sync.dma_start(idxflat[:].rearrange("(p f) -> p f", p=16), idfK[:])
                cwflat2 = dp.tile([C], dt.bfloat16)
                nc.sync.dma_start(
                    cwflat2[:].rearrange("(p f) -> p f", p=16),
                    pc_cw2[:].bitcast(dt.bfloat16),
                )

                nc.sync.dma_start(ids128[:], idxflat[:].rearrange("(j p) -> p j", p=P))
                cw128b = sm.tile([P, CT], dt.bfloat16)
                nc.sync.dma_start(cw128b[:], cwflat2[:].rearrange("(j p) -> p j", p=P))
                nc.vector.tensor_copy(cw128[:], cw128b[:])

                nc.vector.tensor_copy(idx_i[:], ids128[:])
                nc.sync.dma_start(idx_out[:].rearrange("(j p) -> p j", p=P), idx_i[:])
                idg_f = sm.tile([P, CT], dt.float32)
                nc.vector.tensor_scalar_min(idg_f[:], ids128[:], float(T - 1))
                nc.vector.tensor_copy(idg_i[:], idg_f[:])

            # ---------------- expert FFN on compact tokens (bf16) ----------------
            with (
                tc.tile_pool(name="f_gx", bufs=4) as fgx,
                tc.tile_pool(name="f_xT", bufs=1) as fxt,
                tc.tile_pool(name="f_hT", bufs=1) as fht,
                tc.tile_pool(name="f_w", bufs=3) as fw,
                tc.tile_pool(name="f_misc", bufs=2) as fm,
            ):
                jt0 = 0
                for tc_size in CHUNKS:
                    nt = tc_size // P
                    nsub = tc_size // 512 if tc_size % 512 == 0 else 1
                    subs = (
                        [(o, 512) for o in range(0, tc_size, 512)]
                        if tc_size % 512 == 0
                        else [(0, tc_size)]
                    )

                    xTall = fxt.tile([P, HC, tc_size], dt.bfloat16, tag="xT")
                    with tc.tile_pool(name="ps_t", bufs=1, space="PSUM") as pt:
                        for jj in range(nt):
                            gx = fgx.tile([P, H], dt.bfloat16, tag="gx")
                            nc.gpsimd.indirect_dma_start(
                                out=gx[:],
                                out_offset=None,
                                in_=xb_in[:],
                                in_offset=IndirectOffsetOnAxis(
                                    ap=idg_i[:, jt0 + jj : jt0 + jj + 1], axis=0
                                ),
                            )
                            ptf = pt.tile([P, HC, P], dt.bfloat16, tag="ft", bufs=2)
                            for hc in range(HC):
                                nc.tensor.transpose(
                                    ptf[:, hc, :], gx[:, hc * P : (hc + 1) * P],
                                    identb[:],
                                )
                            nc.vector.tensor_copy(
                                xTall[:, :, jj * P : (jj + 1) * P], ptf[:]
                            )

                    # hT[f, tok] = silu(x@w1) * (x@w3), stored bf16
                    hT = fht.tile([P, FT, tc_size], dt.bfloat16, tag="hT")
                    with tc.tile_pool(name="ps_h", bufs=1, space="PSUM") as ph:
                        for ft in range(FT):
                            w1t = fw.tile([P, HC, P], dt.bfloat16, tag="w1")
                            nc.sync.dma_start(w1t[:], w1_in[ft])
                            w3t = fw.tile([P, HC, P], dt.bfloat16, tag="w3")
                            nc.scalar.dma_start(w3t[:], w3_in[ft])
                            pa = ph.tile([P, tc_size], dt.float32, tag="pa", bufs=2)
                            pb = ph.tile([P, tc_size], dt.float32, tag="pb", bufs=2)
                            for hc in range(HC):
                                for off, sz in subs:
                                    nc.tensor.matmul(
                                        pa[:, off : off + sz], w1t[:, hc, :],
                                        xTall[:, hc, off : off + sz],
                                        start=(hc == 0), stop=(hc == HC - 1),
                                        skip_group_check=True,
                                    )
                                for off, sz in subs:
                                    nc.tensor.matmul(
                                        pb[:, off : off + sz], w3t[:, hc, :],
                                        xTall[:, hc, off : off + sz],
                                        start=(hc == 0), stop=(hc == HC - 1),
                                        skip_group_check=True,
                                    )
                            sl = fm.tile([P, tc_size], dt.float32, tag="sl")
                            nc.scalar.activation(sl[:], pa[:], Act.Silu)
                            nc.vector.tensor_tensor(
                                hT[:, ft, :], sl[:], pb[:], op=Alu.mult
                            )

                    # y[tok, h] = hT.T @ w2, PSUM-accumulated over 32 F-blocks;
                    # each hT stationary serves both 512-col H halves.
                    with tc.tile_pool(name="ps_y", bufs=1, space="PSUM") as pyp:
                        for ts in range(nt):
                            py = pyp.tile([P, H], dt.float32, tag="py", bufs=2)
                            for fb in range(FT):
                                for hn in range(2):
                                    nc.tensor.matmul(
                                        py[:, hn * 512 : (hn + 1) * 512],
                                        hT[:, fb, ts * P : (ts + 1) * P],
                                        w2r[:, fb, hn * 512 : (hn + 1) * 512],
                                        start=(fb == 0), stop=(fb == FT - 1),
                                        skip_group_check=True,
                                    )
                            ysb = fm.tile([P, H], dt.float32, tag="ysb")
                            nc.vector.tensor_scalar(
                                ysb[:], py[:],
                                cw128[:, jt0 + ts : jt0 + ts + 1], None,
                                op0=Alu.mult,
                            )
                            nc.sync.dma_start(
                                y_out[:].rearrange("(a p) h -> p a h", p=P)[
                                    :, jt0 + ts, :
                                ],
                                ysb[:],
                            )
                    jt0 += nt

    nc.finalize()
    return nc


def _prep_shared(xf, gate_w, w1, w2, w3):
    """Host prep shared across cores (x transposes/casts)."""
    import ml_dtypes

    bf16 = ml_dtypes.bfloat16
    xt = np.ascontiguousarray(
        xf.reshape(NBLK, 512, HC, P).transpose(0, 3, 2, 1)
    )
    xb = xf.astype(bf16)
    gwt = np.ascontiguousarray(
        gate_w.T.reshape(HC, P, E).transpose(1, 0, 2)
    ).astype(np.float32)
    return xt, xb, gwt


def _prep_core_inputs(xf, gate_w, w1, w2, w3, e, shared=None):
    import ml_dtypes

    bf16 = ml_dtypes.bfloat16
    if shared is None:
        shared = _prep_shared(xf, gate_w, w1, w2, w3)
    xt, xb, gwt = shared
    esel = np.zeros((P, E), dtype=np.float32)
    esel[:, e] = 1.0
    w1t = np.ascontiguousarray(
        w1[e].astype(bf16).reshape(HC, P, FT, P).transpose(2, 1, 0, 3)
    )
    w3t = np.ascontiguousarray(
        w3[e].astype(bf16).reshape(HC, P, FT, P).transpose(2, 1, 0, 3)
    )
    w2n = w2[e].astype(bf16).reshape(FT, P, H)
    return {
        "xt": xt, "xb": xb, "gwt": gwt, "esel": esel,
        "w1t": w1t, "w3t": w3t, "w2n": w2n,
    }


def _run(inputs, trace=False):
    from concourse.bass_utils import run_bass_kernel_spmd

    x = np.ascontiguousarray(np.asarray(inputs["x"], dtype=np.float32))
    gate_w = np.ascontiguousarray(np.asarray(inputs["gate_w"], dtype=np.float32))
    w1 = np.ascontiguousarray(np.asarray(inputs["w1"], dtype=np.float32))
    w2 = np.ascontiguousarray(np.asarray(inputs["w2"], dtype=np.float32))
    w3 = np.ascontiguousarray(np.asarray(inputs["w3"], dtype=np.float32))
    xf = x.reshape(T, H)

    # capacity safety check for both compaction passes (host-side routing
    # estimate; K/K2 carry margin over this estimate's tie-break uncertainty)
    logits = xf @ gate_w.T
    m2 = np.sort(logits, axis=1)[:, -2:-1]
    mask = logits >= m2
    pp = mask.reshape(16, 512, E).sum(axis=1)
    if pp.max() > K:
        raise RuntimeError(
            f"pass-1 per-partition expert token count {pp.max()} exceeds "
            f"compiled capacity K={K}; rebuild kernel.py with a larger K"
        )
    s = np.arange(16 * K)
    for e in range(E):
        valid = np.zeros(16 * K, bool)
        for p in range(16):
            valid[p * K : p * K + pp[p, e]] = True
        c2 = np.bincount((s % 16)[valid], minlength=16)
        if c2.max() > K2:
            raise RuntimeError(
                f"pass-2 per-partition count {c2.max()} (expert {e}) exceeds "
                f"compiled capacity K2={K2}; rebuild kernel.py with larger K2"
            )

    if "nc" not in _cache:
        _cache["nc"] = _build_nc()
    nc = _cache["nc"]

    shared = _prep_shared(xf, gate_w, w1, w2, w3)
    in_maps = [
        _prep_core_inputs(xf, gate_w, w1, w2, w3, e, shared) for e in range(NCORES)
    ]
    res = run_bass_kernel_spmd(nc, in_maps, core_ids=list(range(NCORES)), trace=trace)

    out = np.zeros((T + 1, H), dtype=np.float32)
    for e in range(NCORES):
        idx = res.results[e]["idx"]
        y = res.results[e]["y"]
        out[idx] += y
    return out[:T].reshape(x.shape), res


def kernel(**inputs) -> np.ndarray:
    out, _ = _run(inputs, trace=False)
    return out


# revision 19
# speedup vs baseline: 1.0697x; 1.0697x over previous
"""MoE layer (top-2 of 8 experts) on 8 Trainium2 NeuronCores, expert-parallel.

v4: restructured for real-HW matmul cost (stationary weight loads are NOT
hidden: ~125ns per reload, so each stationary must serve >=1024 moving
columns):
 - routing uses host-pretransposed xT; gate logits computed as
   [8 experts x tokens] with gate weights stationary (no PE transposes of x),
   then tiny [8->128] transposes recover per-token layout.
 - FFN w1/w3: hc-outer matmul pairs, each stationary serves 1024 cols.
 - FFN w2: per (hT-block, token-tile) stationary serves both 512-col H
   halves; w2 fully resident in SBUF (loaded once).
 - compaction: combine weights scattered as single bf16 local_scatter, then
   a second round-robin re-deal + re-scatter rebalances partitions
   (peak per-partition count 157 -> 142), shrinking the padded compact
   token count C from 2560 to 2304 (-10% FFN matmul work).
Routing math stays fp32 (exact top-2 selection); FFN is bf16 with fp32
PSUM accumulation.
"""

import os

os.environ.setdefault("JAX_PLATFORMS", "")

import numpy as np

T, H, F, E = 8192, 1024, 4096, 8
P = 128
NCORES = 8
K = 160                      # pass-1 per-partition slot capacity ([16, 512] layout)
K2 = 144                     # pass-2 capacity after round-robin rebalance
C = 16 * K2                  # 2304 compact slots per expert
CT = C // P                  # 18 slot tiles
CHUNKS = [1024, 1024, 256]   # FFN token chunks
assert sum(CHUNKS) == C
NTILE = T // P               # 64 routing tiles
NBLK = 16                    # routing blocks of 512 tokens
HC = H // P                  # 8 h-blocks
FT = F // P                  # 32 f-blocks

_cache: dict = {}


def _build_nc(reps=1):
    import concourse.mybir as mybir
    import concourse.tile as tile
    from concourse import bacc
    from concourse.bass import IndirectOffsetOnAxis
    from concourse.masks import make_identity

    dt = mybir.dt
    Alu = mybir.AluOpType
    Act = mybir.ActivationFunctionType

    nc = bacc.Bacc("TRN2", target_bir_lowering=False)

    xt_in = nc.dram_tensor("xt", [NBLK, P, HC, 512], dt.float32, kind="ExternalInput")
    xb_in = nc.dram_tensor("xb", [T, H], dt.bfloat16, kind="ExternalInput")
    gwt_in = nc.dram_tensor("gwt", [P, HC, E], dt.float32, kind="ExternalInput")
    esel_in = nc.dram_tensor("esel", [P, E], dt.float32, kind="ExternalInput")
    w1_in = nc.dram_tensor("w1t", [FT, P, HC, P], dt.bfloat16, kind="ExternalInput")
    w3_in = nc.dram_tensor("w3t", [FT, P, HC, P], dt.bfloat16, kind="ExternalInput")
    w2_in = nc.dram_tensor("w2n", [FT, P, H], dt.bfloat16, kind="ExternalInput")

    y_out = nc.dram_tensor("y", [C, H], dt.float32, kind="ExternalOutput")
    idx_out = nc.dram_tensor("idx", [C], dt.int32, kind="ExternalOutput")

    with tile.TileContext(nc) as tc:
      for _rep in range(reps):
        with (
            tc.tile_pool(name="const", bufs=1) as cp,
            tc.tile_pool(name="dram", bufs=1, space="DRAM") as dp,
        ):
            identb = cp.tile([P, P], dt.bfloat16)
            make_identity(nc, identb)
            ident8 = cp.tile([8, 8], dt.float32)
            make_identity(nc, ident8)
            gwt = cp.tile([P, HC, E], dt.float32)
            nc.sync.dma_start(gwt[:], gwt_in[:])
            esel = cp.tile([P, E], dt.float32)
            nc.sync.dma_start(esel[:], esel_in[:])

            # w2 resident for the whole FFN (loaded during compaction/hT;
            # issued after the routing loads so it doesn't contend with them)
            w2r = cp.tile([P, FT, H], dt.bfloat16)

            # routing result: cw per token, layout [p, i] -> t = i*128+p
            cw_all = cp.tile([P, NTILE], dt.float32)

            # ---------------- routing (all 8192 tokens) ----------------
            with (
                tc.tile_pool(name="rt_x", bufs=3) as rx,
                tc.tile_pool(name="rt_misc", bufs=3) as rm,
                tc.tile_pool(name="ps_rt", bufs=1, space="PSUM") as pr,
            ):
                for b in range(NBLK):
                    xtb = rx.tile([P, HC, 512], dt.float32, tag="xtb")
                    (nc.sync if b % 2 == 0 else nc.scalar).dma_start(
                        xtb[:], xt_in[b]
                    )
                    pg = pr.tile([8, 512], dt.float32, tag="pg", bufs=2)
                    for hc in range(HC):
                        nc.tensor.matmul(
                            pg[:], gwt[:, hc, :], xtb[:, hc, :],
                            start=(hc == 0), stop=(hc == HC - 1),
                        )
                    lgE = rm.tile([8, 512], dt.float32, tag="lgE")
                    nc.vector.tensor_copy(lgE[:], pg[:])
                    for s in range(4):
                        i = b * 4 + s  # 128-token tile index
                        tr = pr.tile([P, 8], dt.float32, tag="tr", bufs=2)
                        nc.tensor.transpose(
                            tr[:], lgE[:, s * P : (s + 1) * P], ident8[:]
                        )
                        mx = rm.tile([P, 8], dt.float32, tag="mx")
                        nc.vector.max(mx[:], tr[:])
                        negs = rm.tile([P, 1], dt.float32, tag="negs")
                        nc.vector.tensor_tensor(
                            negs[:], mx[:, 0:1], mx[:, 1:2], op=Alu.add
                        )
                        nc.vector.tensor_scalar_mul(negs[:], negs[:], -1.0)
                        sig = rm.tile([P, E], dt.float32, tag="sig")
                        nc.scalar.activation(
                            sig[:], tr[:], Act.Sigmoid, bias=negs[:], scale=2.0
                        )
                        msk = rm.tile([P, E], dt.float32, tag="msk")
                        nc.vector.tensor_scalar(
                            msk[:], tr[:], mx[:, 1:2], None, op0=Alu.is_ge
                        )
                        cw8 = rm.tile([P, E], dt.float32, tag="cw8")
                        nc.vector.tensor_tensor(cw8[:], sig[:], msk[:], op=Alu.mult)
                        nc.vector.tensor_tensor(cw8[:], cw8[:], esel[:], op=Alu.mult)
                        nc.vector.tensor_reduce(
                            cw_all[:, i : i + 1], cw8[:],
                            axis=mybir.AxisListType.X, op=Alu.add,
                        )

            for fb in range(FT):
                nc.scalar.dma_start(w2r[:, fb, :], w2_in[fb])

            # -------- compaction: [16,512] layout, scan + local_scatter --------
            ids128 = cp.tile([P, CT], dt.float32)
            cw128 = cp.tile([P, CT], dt.float32)
            idx_i = cp.tile([P, CT], dt.int32)
            idg_i = cp.tile([P, CT], dt.int32)
            with tc.tile_pool(name="cmp", bufs=1) as sm:
                cwflat = dp.tile([T], dt.float32)
                nc.sync.dma_start(cwflat[:].rearrange("(i p) -> p i", p=P), cw_all[:])
                cw16 = sm.tile([16, 512], dt.float32)
                nc.sync.dma_start(cw16[:], cwflat[:].rearrange("(p f) -> p f", p=16))

                mask16 = sm.tile([16, 512], dt.float32)
                nc.vector.tensor_scalar(mask16[:], cw16[:], 0.0, None, op0=Alu.is_gt)
                zeros16 = sm.tile([16, 512], dt.float32)
                nc.vector.memset(zeros16[:], 0.0)
                scn = sm.tile([16, 512], dt.float32)
                nc.vector.tensor_tensor_scan(
                    scn[:], mask16[:], zeros16[:], 0.0, Alu.add, Alu.add
                )
                pos = sm.tile([16, 512], dt.float32)
                nc.vector.tensor_tensor(pos[:], scn[:], mask16[:], op=Alu.subtract)
                inb = sm.tile([16, 512], dt.float32)
                nc.vector.tensor_scalar(inb[:], pos[:], float(K - 1), None, op0=Alu.is_le)
                sel = sm.tile([16, 512], dt.float32)
                nc.vector.tensor_tensor(sel[:], mask16[:], inb[:], op=Alu.mult)
                posf = sm.tile([16, 512], dt.float32)
                nc.vector.tensor_tensor(posf[:], pos[:], sel[:], op=Alu.mult)
                selm1 = sm.tile([16, 512], dt.float32)
                nc.vector.tensor_scalar(selm1[:], sel[:], 1.0, None, op0=Alu.subtract)
                nc.vector.tensor_tensor(posf[:], posf[:], selm1[:], op=Alu.add)
                posi = sm.tile([16, 512], dt.int16)
                nc.vector.tensor_copy(posi[:], posf[:])

                iop1 = sm.tile([16, 512], dt.int32)
                nc.gpsimd.iota(iop1[:], pattern=[[1, 512]], base=1, channel_multiplier=512)
                idsp1 = sm.tile([16, 512], dt.uint16)
                nc.vector.tensor_copy(idsp1[:], iop1[:])

                # cw as bf16 (single 16-bit scatter; 0.4% rel quantization)
                cwb = sm.tile([16, 512], dt.bfloat16)
                nc.vector.tensor_copy(cwb[:], cw16[:])

                pc_id = sm.tile([16, K], dt.uint16)
                nc.gpsimd.local_scatter(pc_id[:], idsp1[:], posi[:], 16, K, 512)
                pc_cw = sm.tile([16, K], dt.uint16)
                nc.gpsimd.local_scatter(
                    pc_cw[:], cwb[:].bitcast(dt.uint16), posi[:], 16, K, 512
                )

                # ---- pass 2: round-robin re-deal across partitions, then
                # re-compact to K2 (peak per-partition count drops 157->~142
                # because each partition now samples every 16th slot).
                idflat1 = dp.tile([16 * K], dt.uint16)
                nc.sync.dma_start(
                    idflat1[:].rearrange("(p f) -> p f", p=16), pc_id[:]
                )
                cwflat1 = dp.tile([16 * K], dt.uint16)
                nc.sync.dma_start(
                    cwflat1[:].rearrange("(p f) -> p f", p=16), pc_cw[:]
                )
                id_B = sm.tile([16, K], dt.uint16)
                nc.sync.dma_start(id_B[:], idflat1[:].rearrange("(f p) -> p f", p=16))
                cw_B = sm.tile([16, K], dt.uint16)
                nc.sync.dma_start(cw_B[:], cwflat1[:].rearrange("(f p) -> p f", p=16))

                idf2 = sm.tile([16, K], dt.float32)
                nc.vector.tensor_copy(idf2[:], id_B[:])
                vm2 = sm.tile([16, K], dt.float32)
                nc.vector.tensor_scalar(vm2[:], idf2[:], 0.0, None, op0=Alu.is_gt)
                z2 = sm.tile([16, K], dt.float32)
                nc.vector.memset(z2[:], 0.0)
                scn2 = sm.tile([16, K], dt.float32)
                nc.vector.tensor_tensor_scan(
                    scn2[:], vm2[:], z2[:], 0.0, Alu.add, Alu.add
                )
                pos2 = sm.tile([16, K], dt.float32)
                nc.vector.tensor_tensor(pos2[:], scn2[:], vm2[:], op=Alu.subtract)
                inb2 = sm.tile([16, K], dt.float32)
                nc.vector.tensor_scalar(
                    inb2[:], pos2[:], float(K2 - 1), None, op0=Alu.is_le
                )
                sel2 = sm.tile([16, K], dt.float32)
                nc.vector.tensor_tensor(sel2[:], vm2[:], inb2[:], op=Alu.mult)
                posf2 = sm.tile([16, K], dt.float32)
                nc.vector.tensor_tensor(posf2[:], pos2[:], sel2[:], op=Alu.mult)
                sm2 = sm.tile([16, K], dt.float32)
                nc.vector.tensor_scalar(sm2[:], sel2[:], 1.0, None, op0=Alu.subtract)
                nc.vector.tensor_tensor(posf2[:], posf2[:], sm2[:], op=Alu.add)
                posi2 = sm.tile([16, K], dt.int16)
                nc.vector.tensor_copy(posi2[:], posf2[:])

                pc_id2 = sm.tile([16, K2], dt.uint16)
                nc.gpsimd.local_scatter(pc_id2[:], id_B[:], posi2[:], 16, K2, K)
                pc_cw2 = sm.tile([16, K2], dt.uint16)
                nc.gpsimd.local_scatter(pc_cw2[:], cw_B[:], posi2[:], 16, K2, K)

                idfK = sm.tile([16, K2], dt.float32)
                nc.vector.tensor_copy(idfK[:], pc_id2[:])
                zt = sm.tile([16, K2], dt.float32)
                nc.vector.tensor_scalar(
                    zt[:], idfK[:], 0.0, 8193.0, op0=Alu.is_equal, op1=Alu.mult
                )
                nc.vector.tensor_tensor(idfK[:], idfK[:], zt[:], op=Alu.add)
                nc.vector.tensor_scalar(idfK[:], idfK[:], 1.0, None, op0=Alu.subtract)

                idxflat = dp.tile([C], dt.float32)
                nc.sync.dma_start(idxflat[:].rearrange("(p f) -> p f", p=16), idfK[:])
                cwflat2 = dp.tile([C], dt.bfloat16)
                nc.sync.dma_start(
                    cwflat2[:].rearrange("(p f) -> p f", p=16),
                    pc_cw2[:].bitcast(dt.bfloat16),
                )

                nc.sync.dma_start(ids128[:], idxflat[:].rearrange("(j p) -> p j", p=P))
                cw128b = sm.tile([P, CT], dt.bfloat16)
                nc.sync.dma_start(cw128b[:], cwflat2[:].rearrange("(j p) -> p j", p=P))
                nc.vector.tensor_copy(cw128[:], cw128b[:])

                nc.vector.tensor_copy(idx_i[:], ids128[:])
                nc.sync.dma_start(idx_out[:].rearrange("(j p) -> p j", p=P), idx_i[:])
                idg_f = sm.tile([P, CT], dt.float32)
                nc.vector.tensor_scalar_min(idg_f[:], ids128[:], float(T - 1))
                nc.vector.tensor_copy(idg_i[:], idg_f[:])

            # ---------------- expert FFN on compact tokens (bf16) ----------------
            with (
                tc.tile_pool(name="f_gx", bufs=4) as fgx,
                tc.tile_pool(name="f_xT", bufs=1) as fxt,
                tc.tile_pool(name="f_hT", bufs=1) as fht,
                tc.tile_pool(name="f_w", bufs=3) as fw,
                tc.tile_pool(name="f_misc", bufs=2) as fm,
            ):
                jt0 = 0
                for tc_size in CHUNKS:
                    nt = tc_size // P
                    nsub = tc_size // 512 if tc_size % 512 == 0 else 1
                    subs = (
                        [(o, 512) for o in range(0, tc_size, 512)]
                        if tc_size % 512 == 0
                        else [(0, tc_size)]
                    )

                    xTall = fxt.tile([P, HC, tc_size], dt.bfloat16, tag="xT")
                    with tc.tile_pool(name="ps_t", bufs=1, space="PSUM") as pt:
                        for jj in range(nt):
                            gx = fgx.tile([P, H], dt.bfloat16, tag="gx")
                            nc.gpsimd.indirect_dma_start(
                                out=gx[:],
                                out_offset=None,
                                in_=xb_in[:],
                                in_offset=IndirectOffsetOnAxis(
                                    ap=idg_i[:, jt0 + jj : jt0 + jj + 1], axis=0
                                ),
                            )
                            ptf = pt.tile([P, HC, P], dt.bfloat16, tag="ft", bufs=2)
                            for hc in range(HC):
                                nc.tensor.transpose(
                                    ptf[:, hc, :], gx[:, hc * P : (hc + 1) * P],
                                    identb[:],
                                )
                            nc.vector.tensor_copy(
                                xTall[:, :, jj * P : (jj + 1) * P], ptf[:]
                            )

                    # hT[f, tok] = silu(x@w1) * (x@w3), stored bf16
                    hT = fht.tile([P, FT, tc_size], dt.bfloat16, tag="hT")
                    with tc.tile_pool(name="ps_h", bufs=1, space="PSUM") as ph:
                        for ft in range(FT):
                            w1t = fw.tile([P, HC, P], dt.bfloat16, tag="w1")
                            nc.sync.dma_start(w1t[:], w1_in[ft])
                            w3t = fw.tile([P, HC, P], dt.bfloat16, tag="w3")
                            nc.scalar.dma_start(w3t[:], w3_in[ft])
                            pa = ph.tile([P, tc_size], dt.float32, tag="pa", bufs=2)
                            pb = ph.tile([P, tc_size], dt.float32, tag="pb", bufs=2)
                            for hc in range(HC):
                                for off, sz in subs:
                                    nc.tensor.matmul(
                                        pa[:, off : off + sz], w1t[:, hc, :],
                                        xTall[:, hc, off : off + sz],
                                        start=(hc == 0), stop=(hc == HC - 1),
                                        skip_group_check=True,
                                    )
                                for off, sz in subs:
                                    nc.tensor.matmul(
                                        pb[:, off : off + sz], w3t[:, hc, :],
                                        xTall[:, hc, off : off + sz],
                                        start=(hc == 0), stop=(hc == HC - 1),
                                        skip_group_check=True,
                                    )
                            sl = fm.tile([P, tc_size], dt.float32, tag="sl")
                            nc.scalar.activation(sl[:], pa[:], Act.Silu)
                            nc.vector.tensor_tensor(
                                hT[:, ft, :], sl[:], pb[:], op=Alu.mult
                            )

                    # y[tok, h] = hT.T @ w2, PSUM-accumulated over 32 F-blocks;
                    # each hT stationary serves both 512-col H halves.
                    with tc.tile_pool(name="ps_y", bufs=1, space="PSUM") as pyp:
                        for ts in range(nt):
                            py = pyp.tile([P, H], dt.float32, tag="py", bufs=2)
                            for fb in range(FT):
                                for hn in range(2):
                                    nc.tensor.matmul(
                                        py[:, hn * 512 : (hn + 1) * 512],
                                        hT[:, fb, ts * P : (ts + 1) * P],
                                        w2r[:, fb, hn * 512 : (hn + 1) * 512],
                                        start=(fb == 0), stop=(fb == FT - 1),
                                        skip_group_check=True,
                                    )
                            ysb = fm.tile([P, H], dt.float32, tag="ysb")
                            nc.vector.tensor_scalar(
                                ysb[:], py[:],
                                cw128[:, jt0 + ts : jt0 + ts + 1], None,
                                op0=Alu.mult,
                            )
                            nc.sync.dma_start(
                                y_out[:].rearrange("(a p) h -> p a h", p=P)[
                                    :, jt0 + ts, :
                                ],
                                ysb[:],
                            )
                    jt0 += nt

    nc.finalize()
    return nc


def _prep_shared(xf, gate_w, w1, w2, w3):
    """Host prep shared across cores (x transposes/casts)."""
    import ml_dtypes

    bf16 = ml_dtypes.bfloat16
    xt = np.ascontiguousarray(
        xf.reshape(NBLK, 512, HC, P).transpose(0, 3, 2, 1)
    )
    xb = xf.astype(bf16)
    gwt = np.ascontiguousarray(
        gate_w.T.reshape(HC, P, E).transpose(1, 0, 2)
    ).astype(np.float32)
    return xt, xb, gwt


def _prep_core_inputs(xf, gate_w, w1, w2, w3, e, shared=None):
    import ml_dtypes

    bf16 = ml_dtypes.bfloat16
    if shared is None:
        shared = _prep_shared(xf, gate_w, w1, w2, w3)
    xt, xb, gwt = shared
    esel = np.zeros((P, E), dtype=np.float32)
    esel[:, e] = 1.0
    w1t = np.ascontiguousarray(
        w1[e].astype(bf16).reshape(HC, P, FT, P).transpose(2, 1, 0, 3)
    )
    w3t = np.ascontiguousarray(
        w3[e].astype(bf16).reshape(HC, P, FT, P).transpose(2, 1, 0, 3)
    )
    w2n = w2[e].astype(bf16).reshape(FT, P, H)
    return {
        "xt": xt, "xb": xb, "gwt": gwt, "esel": esel,
        "w1t": w1t, "w3t": w3t, "w2n": w2n,
    }


def _run(inputs, trace=False):
    from concourse.bass_utils import run_bass_kernel_spmd

    x = np.ascontiguousarray(np.asarray(inputs["x"], dtype=np.float32))
    gate_w = np.ascontiguousarray(np.asarray(inputs["gate_w"], dtype=np.float32))
    w1 = np.ascontiguousarray(np.asarray(inputs["w1"], dtype=np.float32))
    w2 = np.ascontiguousarray(np.asarray(inputs["w2"], dtype=np.float32))
    w3 = np.ascontiguousarray(np.asarray(inputs["w3"], dtype=np.float32))
    xf = x.reshape(T, H)

    # capacity safety check for both compaction passes (host-side routing
    # estimate; K/K2 carry margin over this estimate's tie-break uncertainty)
    logits = xf @ gate_w.T
    m2 = np.sort(logits, axis=1)[:, -2:-1]
    mask = logits >= m2
    pp = mask.reshape(16, 512, E).sum(axis=1)
    if pp.max() > K:
        raise RuntimeError(
            f"pass-1 per-partition expert token count {pp.max()} exceeds "
            f"compiled capacity K={K}; rebuild kernel.py with a larger K"
        )
    s = np.arange(16 * K)
    for e in range(E):
        valid = np.zeros(16 * K, bool)
        for p in range(16):
            valid[p * K : p * K + pp[p, e]] = True
        c2 = np.bincount((s % 16)[valid], minlength=16)
        if c2.max() > K2:
            raise RuntimeError(
                f"pass-2 per-partition count {c2.max()} (expert {e}) exceeds "
                f"compiled capacity K2={K2}; rebuild kernel.py with larger K2"
            )

    if "nc" not in _cache:
        _cache["nc"] = _build_nc()
    nc = _cache["nc"]

    shared = _prep_shared(xf, gate_w, w1, w2, w3)
    in_maps = [
        _prep_core_inputs(xf, gate_w, w1, w2, w3, e, shared) for e in range(NCORES)
    ]
    res = run_bass_kernel_spmd(nc, in_maps, core_ids=list(range(NCORES)), trace=trace)

    out = np.zeros((T + 1, H), dtype=np.float32)
    for e in range(NCORES):
        idx = res.results[e]["idx"]
        y = res.results[e]["y"]
        out[idx] += y
    return out[:T].reshape(x.shape), res


def kernel(**inputs) -> np.ndarray:
    out, _ = _run(inputs, trace=False)
    return out


# revision 20
# speedup vs baseline: 1.2770x; 1.1938x over previous
"""MoE layer (top-2 of 8 experts) on 8 Trainium2 NeuronCores, expert-parallel.

v4: restructured for real-HW matmul cost (stationary weight loads are NOT
hidden: ~125ns per reload, so each stationary must serve >=1024 moving
columns):
 - routing uses host-pretransposed xT; gate logits computed as
   [8 experts x tokens] with gate weights stationary (no PE transposes of x),
   then tiny [8->128] transposes recover per-token layout.
 - FFN w1/w3: hc-outer matmul pairs, each stationary serves 1024 cols.
 - FFN w2: per (hT-block, token-tile) stationary serves both 512-col H
   halves; w2 fully resident in SBUF (loaded once).
 - compaction: combine weights scattered as single bf16 local_scatter, then
   a second round-robin re-deal + re-scatter rebalances partitions
   (peak per-partition count 157 -> 142), shrinking the padded compact
   token count C from 2560 to 2304 (-10% FFN matmul work).
Routing math stays fp32 (exact top-2 selection); FFN is bf16 with fp32
PSUM accumulation. Routing x loads alternate between the SP and ACT DMA
queues (one contiguous 2MB block each), and the 8MB w2 preload is issued
after them so it drains during compaction instead of contending with the
DMA-bound routing prologue.
"""

import os

os.environ.setdefault("JAX_PLATFORMS", "")

import numpy as np

T, H, F, E = 8192, 1024, 4096, 8
P = 128
NCORES = 8
K = 160                      # pass-1 per-partition slot capacity ([16, 512] layout)
K2 = 144                     # pass-2 capacity after round-robin rebalance
C = 16 * K2                  # 2304 compact slots per expert
CT = C // P                  # 18 slot tiles
CHUNKS = [1024, 1024, 256]   # FFN token chunks
assert sum(CHUNKS) == C
NTILE = T // P               # 64 routing tiles
NBLK = 16                    # routing blocks of 512 tokens
HC = H // P                  # 8 h-blocks
FT = F // P                  # 32 f-blocks

_cache: dict = {}


def _build_nc(reps=1):
    import concourse.mybir as mybir
    import concourse.tile as tile
    from concourse import bacc
    from concourse.bass import IndirectOffsetOnAxis
    from concourse.masks import make_identity

    dt = mybir.dt
    Alu = mybir.AluOpType
    Act = mybir.ActivationFunctionType

    nc = bacc.Bacc("TRN2", target_bir_lowering=False)

    xt_in = nc.dram_tensor("xt", [NBLK, P, HC, 512], dt.float32, kind="ExternalInput")
    xb_in = nc.dram_tensor("xb", [T, H], dt.bfloat16, kind="ExternalInput")
    gwt_in = nc.dram_tensor("gwt", [P, HC, E], dt.float32, kind="ExternalInput")
    esel_in = nc.dram_tensor("esel", [P, E], dt.float32, kind="ExternalInput")
    w1_in = nc.dram_tensor("w1t", [FT, P, HC, P], dt.bfloat16, kind="ExternalInput")
    w3_in = nc.dram_tensor("w3t", [FT, P, HC, P], dt.bfloat16, kind="ExternalInput")
    w2_in = nc.dram_tensor("w2n", [FT, P, H], dt.bfloat16, kind="ExternalInput")

    y_out = nc.dram_tensor("y", [C, H], dt.float32, kind="ExternalOutput")
    idx_out = nc.dram_tensor("idx", [C], dt.int32, kind="ExternalOutput")

    with tile.TileContext(nc) as tc:
      for _rep in range(reps):
        with (
            tc.tile_pool(name="const", bufs=1) as cp,
            tc.tile_pool(name="dram", bufs=1, space="DRAM") as dp,
        ):
            identb = cp.tile([P, P], dt.bfloat16)
            make_identity(nc, identb)
            ident8 = cp.tile([8, 8], dt.float32)
            make_identity(nc, ident8)
            gwt = cp.tile([P, HC, E], dt.float32)
            nc.sync.dma_start(gwt[:], gwt_in[:])
            esel = cp.tile([P, E], dt.float32)
            nc.sync.dma_start(esel[:], esel_in[:])

            # w2 resident for the whole FFN (loaded during compaction/hT;
            # issued after the routing loads so it doesn't contend with them)
            w2r = cp.tile([P, FT, H], dt.bfloat16)

            # routing result: cw per token, layout [p, i] -> t = i*128+p
            cw_all = cp.tile([P, NTILE], dt.float32)

            # ---------------- routing (all 8192 tokens) ----------------
            with (
                tc.tile_pool(name="rt_x", bufs=3) as rx,
                tc.tile_pool(name="rt_misc", bufs=3) as rm,
                tc.tile_pool(name="ps_rt", bufs=1, space="PSUM") as pr,
            ):
                for b in range(NBLK):
                    xtb = rx.tile([P, HC, 512], dt.float32, tag="xtb")
                    (nc.sync if b % 2 == 0 else nc.scalar).dma_start(
                        xtb[:], xt_in[b]
                    )
                    pg = pr.tile([8, 512], dt.float32, tag="pg", bufs=2)
                    for hc in range(HC):
                        nc.tensor.matmul(
                            pg[:], gwt[:, hc, :], xtb[:, hc, :],
                            start=(hc == 0), stop=(hc == HC - 1),
                        )
                    lgE = rm.tile([8, 512], dt.float32, tag="lgE")
                    nc.vector.tensor_copy(lgE[:], pg[:])
                    for s in range(4):
                        i = b * 4 + s  # 128-token tile index
                        tr = pr.tile([P, 8], dt.float32, tag="tr", bufs=2)
                        nc.tensor.transpose(
                            tr[:], lgE[:, s * P : (s + 1) * P], ident8[:]
                        )
                        mx = rm.tile([P, 8], dt.float32, tag="mx")
                        nc.vector.max(mx[:], tr[:])
                        negs = rm.tile([P, 1], dt.float32, tag="negs")
                        nc.vector.tensor_tensor(
                            negs[:], mx[:, 0:1], mx[:, 1:2], op=Alu.add
                        )
                        nc.vector.tensor_scalar_mul(negs[:], negs[:], -1.0)
                        sig = rm.tile([P, E], dt.float32, tag="sig")
                        nc.scalar.activation(
                            sig[:], tr[:], Act.Sigmoid, bias=negs[:], scale=2.0
                        )
                        msk = rm.tile([P, E], dt.float32, tag="msk")
                        nc.vector.tensor_scalar(
                            msk[:], tr[:], mx[:, 1:2], None, op0=Alu.is_ge
                        )
                        cw8 = rm.tile([P, E], dt.float32, tag="cw8")
                        nc.vector.tensor_tensor(cw8[:], sig[:], msk[:], op=Alu.mult)
                        nc.vector.tensor_tensor(cw8[:], cw8[:], esel[:], op=Alu.mult)
                        nc.vector.tensor_reduce(
                            cw_all[:, i : i + 1], cw8[:],
                            axis=mybir.AxisListType.X, op=Alu.add,
                        )

            for fb in range(FT):
                nc.scalar.dma_start(w2r[:, fb, :], w2_in[fb])

            # -------- compaction: [16,512] layout, scan + local_scatter --------
            ids128 = cp.tile([P, CT], dt.float32)
            cw128 = cp.tile([P, CT], dt.float32)
            idx_i = cp.tile([P, CT], dt.int32)
            idg_i = cp.tile([P, CT], dt.int32)
            with tc.tile_pool(name="cmp", bufs=1) as sm:
                cwflat = dp.tile([T], dt.float32)
                nc.sync.dma_start(cwflat[:].rearrange("(i p) -> p i", p=P), cw_all[:])
                cw16 = sm.tile([16, 512], dt.float32)
                nc.sync.dma_start(cw16[:], cwflat[:].rearrange("(p f) -> p f", p=16))

                mask16 = sm.tile([16, 512], dt.float32)
                nc.vector.tensor_scalar(mask16[:], cw16[:], 0.0, None, op0=Alu.is_gt)
                zeros16 = sm.tile([16, 512], dt.float32)
                nc.vector.memset(zeros16[:], 0.0)
                scn = sm.tile([16, 512], dt.float32)
                nc.vector.tensor_tensor_scan(
                    scn[:], mask16[:], zeros16[:], 0.0, Alu.add, Alu.add
                )
                pos = sm.tile([16, 512], dt.float32)
                nc.vector.tensor_tensor(pos[:], scn[:], mask16[:], op=Alu.subtract)
                inb = sm.tile([16, 512], dt.float32)
                nc.vector.tensor_scalar(inb[:], pos[:], float(K - 1), None, op0=Alu.is_le)
                sel = sm.tile([16, 512], dt.float32)
                nc.vector.tensor_tensor(sel[:], mask16[:], inb[:], op=Alu.mult)
                posf = sm.tile([16, 512], dt.float32)
                nc.vector.tensor_tensor(posf[:], pos[:], sel[:], op=Alu.mult)
                selm1 = sm.tile([16, 512], dt.float32)
                nc.vector.tensor_scalar(selm1[:], sel[:], 1.0, None, op0=Alu.subtract)
                nc.vector.tensor_tensor(posf[:], posf[:], selm1[:], op=Alu.add)
                posi = sm.tile([16, 512], dt.int16)
                nc.vector.tensor_copy(posi[:], posf[:])

                iop1 = sm.tile([16, 512], dt.int32)
                nc.gpsimd.iota(iop1[:], pattern=[[1, 512]], base=1, channel_multiplier=512)
                idsp1 = sm.tile([16, 512], dt.uint16)
                nc.vector.tensor_copy(idsp1[:], iop1[:])

                # cw as bf16 (single 16-bit scatter; 0.4% rel quantization)
                cwb = sm.tile([16, 512], dt.bfloat16)
                nc.vector.tensor_copy(cwb[:], cw16[:])

                pc_id = sm.tile([16, K], dt.uint16)
                nc.gpsimd.local_scatter(pc_id[:], idsp1[:], posi[:], 16, K, 512)
                pc_cw = sm.tile([16, K], dt.uint16)
                nc.gpsimd.local_scatter(
                    pc_cw[:], cwb[:].bitcast(dt.uint16), posi[:], 16, K, 512
                )

                # ---- pass 2: round-robin re-deal across partitions, then
                # re-compact to K2 (peak per-partition count drops 157->~142
                # because each partition now samples every 16th slot).
                idflat1 = dp.tile([16 * K], dt.uint16)
                nc.sync.dma_start(
                    idflat1[:].rearrange("(p f) -> p f", p=16), pc_id[:]
                )
                cwflat1 = dp.tile([16 * K], dt.uint16)
                nc.sync.dma_start(
                    cwflat1[:].rearrange("(p f) -> p f", p=16), pc_cw[:]
                )
                id_B = sm.tile([16, K], dt.uint16)
                nc.sync.dma_start(id_B[:], idflat1[:].rearrange("(f p) -> p f", p=16))
                cw_B = sm.tile([16, K], dt.uint16)
                nc.sync.dma_start(cw_B[:], cwflat1[:].rearrange("(f p) -> p f", p=16))

                idf2 = sm.tile([16, K], dt.float32)
                nc.vector.tensor_copy(idf2[:], id_B[:])
                vm2 = sm.tile([16, K], dt.float32)
                nc.vector.tensor_scalar(vm2[:], idf2[:], 0.0, None, op0=Alu.is_gt)
                z2 = sm.tile([16, K], dt.float32)
                nc.vector.memset(z2[:], 0.0)
                scn2 = sm.tile([16, K], dt.float32)
                nc.vector.tensor_tensor_scan(
                    scn2[:], vm2[:], z2[:], 0.0, Alu.add, Alu.add
                )
                pos2 = sm.tile([16, K], dt.float32)
                nc.vector.tensor_tensor(pos2[:], scn2[:], vm2[:], op=Alu.subtract)
                inb2 = sm.tile([16, K], dt.float32)
                nc.vector.tensor_scalar(
                    inb2[:], pos2[:], float(K2 - 1), None, op0=Alu.is_le
                )
                sel2 = sm.tile([16, K], dt.float32)
                nc.vector.tensor_tensor(sel2[:], vm2[:], inb2[:], op=Alu.mult)
                posf2 = sm.tile([16, K], dt.float32)
                nc.vector.tensor_tensor(posf2[:], pos2[:], sel2[:], op=Alu.mult)
                sm2 = sm.tile([16, K], dt.float32)
                nc.vector.tensor_scalar(sm2[:], sel2[:], 1.0, None, op0=Alu.subtract)
                nc.vector.tensor_tensor(posf2[:], posf2[:], sm2[:], op=Alu.add)
                posi2 = sm.tile([16, K], dt.int16)
                nc.vector.tensor_copy(posi2[:], posf2[:])

                pc_id2 = sm.tile([16, K2], dt.uint16)
                nc.gpsimd.local_scatter(pc_id2[:], id_B[:], posi2[:], 16, K2, K)
                pc_cw2 = sm.tile([16, K2], dt.uint16)
                nc.gpsimd.local_scatter(pc_cw2[:], cw_B[:], posi2[:], 16, K2, K)

                idfK = sm.tile([16, K2], dt.float32)
                nc.vector.tensor_copy(idfK[:], pc_id2[:])
                zt = sm.tile([16, K2], dt.float32)
                nc.vector.tensor_scalar(
                    zt[:], idfK[:], 0.0, 8193.0, op0=Alu.is_equal, op1=Alu.mult
                )
                nc.vector.tensor_tensor(idfK[:], idfK[:], zt[:], op=Alu.add)
                nc.vector.tensor_scalar(idfK[:], idfK[:], 1.0, None, op0=Alu.subtract)

                idxflat = dp.tile([C], dt.float32)
                nc.sync.dma_start(idxflat[:].rearrange("(p f) -> p f", p=16), idfK[:])
                cwflat2 = dp.tile([C], dt.bfloat16)
                nc.sync.dma_start(
                    cwflat2[:].rearrange("(p f) -> p f", p=16),
                    pc_cw2[:].bitcast(dt.bfloat16),
                )

                nc.sync.dma_start(ids128[:], idxflat[:].rearrange("(j p) -> p j", p=P))
                cw128b = sm.tile([P, CT], dt.bfloat16)
                nc.sync.dma_start(cw128b[:], cwflat2[:].rearrange("(j p) -> p j", p=P))
                nc.vector.tensor_copy(cw128[:], cw128b[:])

                nc.vector.tensor_copy(idx_i[:], ids128[:])
                nc.sync.dma_start(idx_out[:].rearrange("(j p) -> p j", p=P), idx_i[:])
                idg_f = sm.tile([P, CT], dt.float32)
                nc.vector.tensor_scalar_min(idg_f[:], ids128[:], float(T - 1))
                nc.vector.tensor_copy(idg_i[:], idg_f[:])

            # ---------------- expert FFN on compact tokens (bf16) ----------------
            with (
                tc.tile_pool(name="f_gx", bufs=4) as fgx,
                tc.tile_pool(name="f_xT", bufs=1) as fxt,
                tc.tile_pool(name="f_hT", bufs=1) as fht,
                tc.tile_pool(name="f_w", bufs=3) as fw,
                tc.tile_pool(name="f_misc", bufs=2) as fm,
            ):
                jt0 = 0
                for tc_size in CHUNKS:
                    nt = tc_size // P
                    nsub = tc_size // 512 if tc_size % 512 == 0 else 1
                    subs = (
                        [(o, 512) for o in range(0, tc_size, 512)]
                        if tc_size % 512 == 0
                        else [(0, tc_size)]
                    )

                    xTall = fxt.tile([P, HC, tc_size], dt.bfloat16, tag="xT")
                    with tc.tile_pool(name="ps_t", bufs=1, space="PSUM") as pt:
                        for jj in range(nt):
                            gx = fgx.tile([P, H], dt.bfloat16, tag="gx")
                            nc.gpsimd.indirect_dma_start(
                                out=gx[:],
                                out_offset=None,
                                in_=xb_in[:],
                                in_offset=IndirectOffsetOnAxis(
                                    ap=idg_i[:, jt0 + jj : jt0 + jj + 1], axis=0
                                ),
                            )
                            ptf = pt.tile([P, HC, P], dt.bfloat16, tag="ft", bufs=2)
                            for hc in range(HC):
                                nc.tensor.transpose(
                                    ptf[:, hc, :], gx[:, hc * P : (hc + 1) * P],
                                    identb[:],
                                )
                            nc.vector.tensor_copy(
                                xTall[:, :, jj * P : (jj + 1) * P], ptf[:]
                            )

                    # hT[f, tok] = silu(x@w1) * (x@w3), stored bf16
                    hT = fht.tile([P, FT, tc_size], dt.bfloat16, tag="hT")
                    with tc.tile_pool(name="ps_h", bufs=1, space="PSUM") as ph:
                        for ft in range(FT):
                            w1t = fw.tile([P, HC, P], dt.bfloat16, tag="w1")
                            nc.sync.dma_start(w1t[:], w1_in[ft])
                            w3t = fw.tile([P, HC, P], dt.bfloat16, tag="w3")
                            nc.scalar.dma_start(w3t[:], w3_in[ft])
                            pa = ph.tile([P, tc_size], dt.float32, tag="pa", bufs=2)
                            pb = ph.tile([P, tc_size], dt.float32, tag="pb", bufs=2)
                            for hc in range(HC):
                                for off, sz in subs:
                                    nc.tensor.matmul(
                                        pa[:, off : off + sz], w1t[:, hc, :],
                                        xTall[:, hc, off : off + sz],
                                        start=(hc == 0), stop=(hc == HC - 1),
                                        skip_group_check=True,
                                    )
                                for off, sz in subs:
                                    nc.tensor.matmul(
                                        pb[:, off : off + sz], w3t[:, hc, :],
                                        xTall[:, hc, off : off + sz],
                                        start=(hc == 0), stop=(hc == HC - 1),
                                        skip_group_check=True,
                                    )
                            sl = fm.tile([P, tc_size], dt.float32, tag="sl")
                            nc.scalar.activation(sl[:], pa[:], Act.Silu)
                            nc.vector.tensor_tensor(
                                hT[:, ft, :], sl[:], pb[:], op=Alu.mult
                            )

                    # y[tok, h] = hT.T @ w2, PSUM-accumulated over 32 F-blocks;
                    # each hT stationary serves both 512-col H halves.
                    with tc.tile_pool(name="ps_y", bufs=1, space="PSUM") as pyp:
                        for ts in range(nt):
                            py = pyp.tile([P, H], dt.float32, tag="py", bufs=2)
                            for fb in range(FT):
                                for hn in range(2):
                                    nc.tensor.matmul(
                                        py[:, hn * 512 : (hn + 1) * 512],
                                        hT[:, fb, ts * P : (ts + 1) * P],
                                        w2r[:, fb, hn * 512 : (hn + 1) * 512],
                                        start=(fb == 0), stop=(fb == FT - 1),
                                        skip_group_check=True,
                                    )
                            ysb = fm.tile([P, H], dt.float32, tag="ysb")
                            nc.vector.tensor_scalar(
                                ysb[:], py[:],
                                cw128[:, jt0 + ts : jt0 + ts + 1], None,
                                op0=Alu.mult,
                            )
                            nc.sync.dma_start(
                                y_out[:].rearrange("(a p) h -> p a h", p=P)[
                                    :, jt0 + ts, :
                                ],
                                ysb[:],
                            )
                    jt0 += nt

    nc.finalize()
    return nc


def _prep_shared(xf, gate_w, w1, w2, w3):
    """Host prep shared across cores (x transposes/casts)."""
    import ml_dtypes

    bf16 = ml_dtypes.bfloat16
    xt = np.ascontiguousarray(
        xf.reshape(NBLK, 512, HC, P).transpose(0, 3, 2, 1)
    )
    xb = xf.astype(bf16)
    gwt = np.ascontiguousarray(
        gate_w.T.reshape(HC, P, E).transpose(1, 0, 2)
    ).astype(np.float32)
    return xt, xb, gwt


def _prep_core_inputs(xf, gate_w, w1, w2, w3, e, shared=None):
    import ml_dtypes

    bf16 = ml_dtypes.bfloat16
    if shared is None:
        shared = _prep_shared(xf, gate_w, w1, w2, w3)
    xt, xb, gwt = shared
    esel = np.zeros((P, E), dtype=np.float32)
    esel[:, e] = 1.0
    w1t = np.ascontiguousarray(
        w1[e].astype(bf16).reshape(HC, P, FT, P).transpose(2, 1, 0, 3)
    )
    w3t = np.ascontiguousarray(
        w3[e].astype(bf16).reshape(HC, P, FT, P).transpose(2, 1, 0, 3)
    )
    w2n = w2[e].astype(bf16).reshape(FT, P, H)
    return {
        "xt": xt, "xb": xb, "gwt": gwt, "esel": esel,
        "w1t": w1t, "w3t": w3t, "w2n": w2n,
    }


def _run(inputs, trace=False):
    from concourse.bass_utils import run_bass_kernel_spmd

    x = np.ascontiguousarray(np.asarray(inputs["x"], dtype=np.float32))
    gate_w = np.ascontiguousarray(np.asarray(inputs["gate_w"], dtype=np.float32))
    w1 = np.ascontiguousarray(np.asarray(inputs["w1"], dtype=np.float32))
    w2 = np.ascontiguousarray(np.asarray(inputs["w2"], dtype=np.float32))
    w3 = np.ascontiguousarray(np.asarray(inputs["w3"], dtype=np.float32))
    xf = x.reshape(T, H)

    # capacity safety check for both compaction passes (host-side routing
    # estimate; K/K2 carry margin over this estimate's tie-break uncertainty)
    logits = xf @ gate_w.T
    m2 = np.sort(logits, axis=1)[:, -2:-1]
    mask = logits >= m2
    pp = mask.reshape(16, 512, E).sum(axis=1)
    if pp.max() > K:
        raise RuntimeError(
            f"pass-1 per-partition expert token count {pp.max()} exceeds "
            f"compiled capacity K={K}; rebuild kernel.py with a larger K"
        )
    s = np.arange(16 * K)
    for e in range(E):
        valid = np.zeros(16 * K, bool)
        for p in range(16):
            valid[p * K : p * K + pp[p, e]] = True
        c2 = np.bincount((s % 16)[valid], minlength=16)
        if c2.max() > K2:
            raise RuntimeError(
                f"pass-2 per-partition count {c2.max()} (expert {e}) exceeds "
                f"compiled capacity K2={K2}; rebuild kernel.py with larger K2"
            )

    if "nc" not in _cache:
        _cache["nc"] = _build_nc()
    nc = _cache["nc"]

    shared = _prep_shared(xf, gate_w, w1, w2, w3)
    in_maps = [
        _prep_core_inputs(xf, gate_w, w1, w2, w3, e, shared) for e in range(NCORES)
    ]
    res = run_bass_kernel_spmd(nc, in_maps, core_ids=list(range(NCORES)), trace=trace)

    out = np.zeros((T + 1, H), dtype=np.float32)
    for e in range(NCORES):
        idx = res.results[e]["idx"]
        y = res.results[e]["y"]
        out[idx] += y
    return out[:T].reshape(x.shape), res


def kernel(**inputs) -> np.ndarray:
    out, _ = _run(inputs, trace=False)
    return out
